# revision 11
# baseline (speedup 1.0000x reference)
"""Bass/Trainium2 kernel for naive causal multi-head attention.

Problem: B=4, S=2048, E=1024, H=16, DH=64 (fp32 in/out).

Sharding (8 NeuronCores): core c handles batch b = c//2 and head group
g = c%2 (heads 8g..8g+7).  Each core computes its 8 heads' attention for
its batch plus the partial out-projection through its 512 columns of the
concat dim; the host sums the two partial outputs per batch.

All device inputs are pre-cast to bf16 and pre-transposed on the host:
  xt     [128, nec, S]  xt[p, ec, s] = x[b, s, 128*ec+p]
  wqkt   [128, HPC, nec, 2*DH]  (q dims 0:64 | k dims 64:128)
  wvt    [128, nec, HD]
  wot    [128, ncc, E]
  maskab [128, 2*1024]  diag-group causal masks

Device-side dataflow (all matmuls bf16, PSUM fp32):
  q/kT = Wqk_h @ xT   -> q_sb[64,S], kt_sb[64,S] per head      (PE)
  v'   = x @ Wv^T (+ ones column per head)                     (PE)
  sT   = kT_blk^T q  -> [128 keys, 512 q] blocks               (PE, causal-skipped)
  p    = exp(sT/8)   (PSUM->SBUF, diag-masked)                 (ACT exp, DVE mask)
  oT   = v'^T p      -> [65, 512]  (row 64 = sums)             (PE, accumulated)
  rec  = 1/sums (from PSUM row 64); bcast via gpsimd           (DVE/POOL)
  cT   = oT * rec                                              (DVE)
  out  = concatT^T @ WoT  -> bf16, host sums the 2 partials    (PE, ACT evac)
"""

import numpy as np
import ml_dtypes

import concourse.bacc as bacc
import concourse.bass as bass
import concourse.mybir as mybir
from concourse.tile import TileContext
from concourse.bass_utils import run_bass_kernel_spmd

F32 = mybir.dt.float32
BF16 = mybir.dt.bfloat16
EXP = mybir.ActivationFunctionType.Exp

N_CORES = 8
BF = ml_dtypes.bfloat16


def build_nc(S=2048, E=1024, HPC=8, DH=64):
    """Build the per-core Bass program (identical on all cores)."""
    NQ = 512                      # query-tile width
    nst = S // 128                # s-tiles (key tiles)
    nec = E // 128                # e chunks (contraction tiles)
    nqt = S // NQ                 # query tiles
    HD = HPC * DH                 # local concat width (512)
    ncc = HD // 128               # concat chunks (4)
    assert NQ == 512 and S % 512 == 0

    nc = bacc.Bacc("TRN2", target_bir_lowering=False, debug=False,
                   num_devices=N_CORES)

    xtd = nc.dram_tensor("xt", [128, nec, S], BF16, kind="ExternalInput")
    wqkt = nc.dram_tensor("wqkt", [128, HPC, nec, 2 * DH], BF16,
                          kind="ExternalInput")
    wvt = nc.dram_tensor("wvt", [128, nec, HD], BF16, kind="ExternalInput")
    wot = nc.dram_tensor("wot", [128, ncc, E], BF16, kind="ExternalInput")
    maskab = nc.dram_tensor("maskab", [128, 2 * 1024], BF16,
                            kind="ExternalInput")
    out = nc.dram_tensor("out", [S, E], BF16, kind="ExternalOutput")

    with TileContext(nc) as tc:
        with (
            tc.tile_pool(name="persist", bufs=1) as persist,
            tc.tile_pool(name="qp", bufs=2) as qp,
            tc.tile_pool(name="kp", bufs=2) as kp,
            tc.tile_pool(name="ptp", bufs=3) as ptp,
            tc.tile_pool(name="recp", bufs=4) as recp,
            tc.tile_pool(name="bcp", bufs=3) as bcp,
            tc.tile_pool(name="outp", bufs=2) as outp,
            tc.tile_pool(name="ps_c1", bufs=2, space="PSUM") as ps_c1,
            tc.tile_pool(name="ps_big", bufs=2, space="PSUM") as ps_big,
            tc.tile_pool(name="ps_o", bufs=2, space="PSUM") as ps_o,
        ):
            # ---- persistent SBUF tensors ----
            xT = persist.tile([128, nec, S], BF16)
            wqk = persist.tile([128, HPC, nec, 2 * DH], BF16)
            wv = persist.tile([128, nec, HD], BF16)
            wo = persist.tile([128, ncc, E], BF16)
            vS = persist.tile([128, nst, HPC * (DH + 1)], BF16)
            cT = persist.tile([128, ncc, S], BF16)
            mk = persist.tile([128, 2 * 1024], BF16)

            # ---- phase A: DMAs ordered so compute starts early ----
            # PE warmup: dummy matmuls during the input-DMA wait keep the
            # HAM clock gate from starting the real stream cold.
            warm = persist.tile([128, 512], BF16)
            nc.vector.memset(warm, 1.0)
            pwarm = ps_c1.tile([128, 512], F32, tag="c1")
            for i in range(32):
                nc.tensor.matmul(pwarm, lhsT=warm[:, 0:128], rhs=warm,
                                 start=(i == 0), stop=(i == 31))
            nc.vector.memset(vS, 1.0)
            nc.sync.dma_start(out=wqk[:, 0], in_=wqkt[:, 0])
            for j in range(nqt):
                nc.sync.dma_start(out=xT[:, :, j * 512:(j + 1) * 512],
                                  in_=xtd[:, :, j * 512:(j + 1) * 512])
                if j == 0:
                    nc.sync.dma_start(out=wv, in_=wvt[:, :, :])
            nc.sync.dma_start(out=wqk[:, 1:HPC], in_=wqkt[:, 1:HPC])
            nc.sync.dma_start(out=mk, in_=maskab[:, :])
            nc.sync.dma_start(out=wo, in_=wot[:, :, :])

            def emit_c1(h, q_sb, kt_sb):
                """q/k projection -> q_sb [64, S], kt_sb [64, S]."""
                for sc in range(nqt):
                    pqk = ps_c1.tile([128, 512], F32, tag="c1")
                    for ec in range(nec):
                        nc.tensor.matmul(
                            pqk, lhsT=wqk[:, h, ec],
                            rhs=xT[:, ec, sc * 512:(sc + 1) * 512],
                            start=(ec == 0), stop=(ec == nec - 1))
                    nc.vector.tensor_copy(
                        out=q_sb[:, sc * 512:(sc + 1) * 512],
                        in_=pqk[0:64, :])
                    nc.vector.tensor_copy(
                        out=kt_sb[:, sc * 512:(sc + 1) * 512],
                        in_=pqk[64:128, :])

            # head 0's projection first: needs only wqk[0] + xT chunks
            qk_tiles = [(qp.tile([64, S], BF16, tag="q", name="q0"),
                         kp.tile([64, S], BF16, tag="kt", name="kt0"))]
            emit_c1(0, *qk_tiles[0])

            # v projection per 128-token tile (stream wv columns)
            for j in range(nqt):
                for st in range(4 * j, 4 * j + 4):
                    pv = ps_c1.tile([128, HD], F32, tag="c1")
                    for ec in range(nec):
                        nc.tensor.matmul(
                            pv, lhsT=xT[:, ec, st * 128:(st + 1) * 128],
                            rhs=wv[:, ec], start=(ec == 0),
                            stop=(ec == nec - 1))
                    nc.vector.tensor_copy(
                        out=vS[:, st].rearrange("p (h m) -> p h m",
                                                m=DH + 1)[:, :, 0:DH],
                        in_=pv.rearrange("p (h m) -> p h m", m=DH))

            # ---- phase C: per head ----
            for h in range(HPC):
                q_sb, kt_sb = qk_tiles[h]

                # C2: attention per query tile
                for qt in range(nqt):
                    po = ps_o.tile([DH + 1, 512], F32, tag="o")
                    ngrp = 2 * qt + 2
                    pts = {}

                    def emit_scores(g):
                        ps2 = ps_big.tile([128, 1024], F32, tag="sbig")
                        for kk in range(2):
                            kt = 2 * g + kk
                            d = kt - 4 * qt
                            n0 = 128 * d if d > 0 else 0
                            nc.tensor.matmul(
                                ps2[:, kk * 512 + n0:(kk + 1) * 512],
                                lhsT=kt_sb[:, kt * 128:(kt + 1) * 128],
                                rhs=q_sb[:, qt * 512 + n0:(qt + 1) * 512],
                                start=True, stop=True)
                        pt = ptp.tile([128, 1024], BF16, tag="pt")
                        nc.scalar.activation(out=pt, in_=ps2, func=EXP,
                                             scale=0.125)
                        if g >= 2 * qt:  # diagonal groups need causal mask
                            mi = g - 2 * qt
                            nc.vector.tensor_mul(
                                pt, pt, mk[:, mi * 1024:(mi + 1) * 1024])
                        pts[g] = pt

                    def emit_av(g):
                        pt = pts.pop(g)
                        for kk in range(2):
                            kt = 2 * g + kk
                            d = kt - 4 * qt
                            n0 = 128 * d if d > 0 else 0
                            nc.tensor.matmul(
                                po[:, n0:512],
                                lhsT=vS[:, kt, h * (DH + 1):(h + 1) * (DH + 1)],
                                rhs=pt[:, kk * 512 + n0:(kk + 1) * 512],
                                start=(g == 0 and kk == 0),
                                stop=(g == ngrp - 1 and kk == 1),
                                skip_group_check=True)

                    # software pipeline: AV one group behind scores, so the
                    # exp/mask chain of group g is covered by scores(g+1)
                    emit_scores(0)
                    for g in range(1, ngrp):
                        emit_scores(g)
                        emit_av(g - 1)
                    emit_av(ngrp - 1)
                    # normalize: rec from PSUM sums row, gpsimd broadcast
                    sums = recp.tile([1, 512], F32, tag="sums")
                    nc.vector.tensor_copy(out=sums, in_=po[DH:DH + 1, :])
                    rec = recp.tile([1, 512], F32, tag="rec")
                    nc.vector.reciprocal_approx_fast(out=rec, in_=sums)
                    bc = bcp.tile([64, 512], F32, tag="bc")
                    nc.gpsimd.partition_broadcast(bc, rec)
                    nc.vector.tensor_mul(
                        cT[64 * (h % 2):64 * (h % 2) + 64, h // 2,
                           qt * 512:(qt + 1) * 512],
                        po[0:DH, :], bc)

                if h + 1 < HPC:  # next head's projection follows its attention
                    qk_tiles.append(
                        (qp.tile([64, S], BF16, tag="q", name=f"q{h + 1}"),
                         kp.tile([64, S], BF16, tag="kt", name=f"kt{h + 1}")))
                    emit_c1(h + 1, *qk_tiles[h + 1])

            # ---- phase D: partial out-projection (bf16 out) ----
            for st in range(nst):
                pd = ps_big.tile([128, E], F32, tag="sbig")
                for c in range(ncc):
                    for n2 in range(2):
                        mm = nc.tensor.matmul(
                            pd[:, n2 * 512:(n2 + 1) * 512],
                            lhsT=cT[:, c, st * 128:(st + 1) * 128],
                            rhs=wo[:, c, n2 * 512:(n2 + 1) * 512],
                            start=(c == 0), stop=(c == ncc - 1),
                            skip_group_check=True)
                        if n2 == 1:  # same stationary as the n2=0 matmul
                            mm.ins.ldweights = False
                osb = outp.tile([128, E], BF16, tag="osb")
                nc.scalar.copy(osb, pd)
                nc.sync.dma_start(out=out[st * 128:(st + 1) * 128, :], in_=osb)

    nc.finalize()
    return nc


def _make_masks(NQ=512):
    """[128, 2*1024] bf16: two diag-group masks (d=0,1 | d=2,3)."""
    j = np.arange(128)[:, None]
    i = np.arange(NQ)[None, :]
    blocks = [(j <= i - 128 * d).astype(np.float32) for d in range(4)]
    mA = np.concatenate(blocks[0:2], axis=1)
    mB = np.concatenate(blocks[2:4], axis=1)
    return np.ascontiguousarray(
        np.concatenate([mA, mB], axis=1)).astype(BF)


def _host_prep(x, Wq, Wk, Wv, Wo, HPC=8, DH=64):
    """Build the 8 per-core input maps (bf16, pre-transposed)."""
    B, S, E = x.shape
    nec = E // 128
    HD = HPC * DH
    masks = _make_masks()
    xts = []
    for b in range(B):
        xt = x[b].T.reshape(nec, 128, S).transpose(1, 0, 2)
        xts.append(np.ascontiguousarray(xt).astype(BF))
    in_maps = []
    for c in range(N_CORES):
        b, g = c // 2, c % 2
        hs = slice(HPC * g, HPC * g + HPC)
        # [h, 2*DH, E] stacked q|k  ->  [128(e%), h, ec, 2*DH]
        wqk = np.concatenate([Wq[hs], Wk[hs]], axis=1)          # [HPC,128,E]
        wqk = wqk.transpose(2, 0, 1).reshape(nec, 128, HPC, 2 * DH)
        wqkt = np.ascontiguousarray(wqk.transpose(1, 2, 0, 3)).astype(BF)
        # Wv slice -> [128, ec, HD]
        wvt = Wv[hs].transpose(2, 0, 1).reshape(nec, 128, HD)
        wvt = np.ascontiguousarray(wvt.transpose(1, 0, 2)).astype(BF)
        # Wo columns slice, transposed -> [128, ncc, E]
        wot = np.ascontiguousarray(Wo[:, HD * g:HD * (g + 1)].T)  # [HD, E]
        wot = np.ascontiguousarray(
            wot.reshape(HD // 128, 128, E).transpose(1, 0, 2)).astype(BF)
        in_maps.append({
            "xt": xts[b],
            "wqkt": wqkt, "wvt": wvt, "wot": wot, "maskab": masks,
        })
    return in_maps


_NC_CACHE = {}


def kernel(x, Wq, Wk, Wv, Wo):
    x = np.asarray(x, dtype=np.float32)
    Wq = np.asarray(Wq, dtype=np.float32)
    Wk = np.asarray(Wk, dtype=np.float32)
    Wv = np.asarray(Wv, dtype=np.float32)
    Wo = np.asarray(Wo, dtype=np.float32)
    B, S, E = x.shape
    H, DH, _ = Wq.shape
    HPC = H // 2

    key = (S, E, HPC, DH)
    if key not in _NC_CACHE:
        _NC_CACHE[key] = build_nc(S=S, E=E, HPC=HPC, DH=DH)
    nc = _NC_CACHE[key]

    in_maps = _host_prep(x, Wq, Wk, Wv, Wo, HPC=HPC, DH=DH)
    res = run_bass_kernel_spmd(nc, in_maps, core_ids=list(range(N_CORES)))
    kernel.last_results = res

    out = np.empty((B, S, E), dtype=np.float32)
    for b in range(B):
        out[b] = (res.results[2 * b]["out"].astype(np.float32)
                  + res.results[2 * b + 1]["out"].astype(np.float32))
    return out


# revision 18
# speedup vs baseline: 1.0030x; 1.0030x over previous
"""Bass/Trainium2 kernel for naive causal multi-head attention.

Problem: B=4, S=2048, E=1024, H=16, DH=64 (fp32 in/out).

Sharding (8 NeuronCores): core c handles batch b = c//2 and head group
g = c%2 (heads 8g..8g+7).  Each core computes its 8 heads' attention for
its batch plus the partial out-projection through its 512 columns of the
concat dim; the host sums the two partial outputs per batch.

All device inputs are pre-cast to bf16 and pre-transposed on the host:
  xt     [128, nec, S]  xt[p, ec, s] = x[b, s, 128*ec+p]
  wqkt   [128, HPC, nec, 2*DH]  (q dims 0:64 | k dims 64:128)
  wvt    [128, nec, HD]
  wot    [128, ncc, E]
  maskab [128, 2*1024]  diag-group causal masks

Device-side dataflow (all matmuls bf16, PSUM fp32):
  q/kT = Wqk_h @ xT   -> q_sb[64,S], kt_sb[64,S] per head      (PE)
  v'   = x @ Wv^T (+ ones column per head)                     (PE)
  sT   = kT_blk^T q  -> [128 keys, 512 q] blocks               (PE, causal-skipped)
  p    = exp(sT/8)   (PSUM->SBUF, diag-masked)                 (ACT exp, DVE mask)
  oT   = v'^T p      -> [65, 512]  (row 64 = sums)             (PE, accumulated)
  rec  = 1/sums (from PSUM row 64); bcast via gpsimd           (DVE/POOL)
  cT   = oT * rec                                              (DVE)
  out  = concatT^T @ WoT  -> bf16, host sums the 2 partials    (PE, ACT evac)
"""

import numpy as np
import ml_dtypes

import concourse.bacc as bacc
import concourse.bass as bass
import concourse.mybir as mybir
from concourse.tile import TileContext
from concourse.bass_utils import run_bass_kernel_spmd

F32 = mybir.dt.float32
BF16 = mybir.dt.bfloat16
EXP = mybir.ActivationFunctionType.Exp

N_CORES = 8
BF = ml_dtypes.bfloat16


def build_nc(S=2048, E=1024, HPC=8, DH=64):
    """Build the per-core Bass program (identical on all cores)."""
    NQ = 512                      # query-tile width
    nst = S // 128                # s-tiles (key tiles)
    nec = E // 128                # e chunks (contraction tiles)
    nqt = S // NQ                 # query tiles
    HD = HPC * DH                 # local concat width (512)
    ncc = HD // 128               # concat chunks (4)
    assert NQ == 512 and S % 512 == 0

    nc = bacc.Bacc("TRN2", target_bir_lowering=False, debug=False,
                   num_devices=N_CORES)

    xtd = nc.dram_tensor("xt", [128, nec, S], BF16, kind="ExternalInput")
    wqkt = nc.dram_tensor("wqkt", [128, HPC, nec, 2 * DH], BF16,
                          kind="ExternalInput")
    wvt = nc.dram_tensor("wvt", [128, nec, HD], BF16, kind="ExternalInput")
    wot = nc.dram_tensor("wot", [128, ncc, E], BF16, kind="ExternalInput")
    maskab = nc.dram_tensor("maskab", [128, 2 * 1024], BF16,
                            kind="ExternalInput")
    out = nc.dram_tensor("out", [S, E], BF16, kind="ExternalOutput")

    with TileContext(nc) as tc:
        with (
            tc.tile_pool(name="persist", bufs=1) as persist,
            tc.tile_pool(name="qp", bufs=2) as qp,
            tc.tile_pool(name="kp", bufs=2) as kp,
            tc.tile_pool(name="ptp", bufs=3) as ptp,
            tc.tile_pool(name="recp", bufs=4) as recp,
            tc.tile_pool(name="bcp", bufs=3) as bcp,
            tc.tile_pool(name="outp", bufs=2) as outp,
            tc.tile_pool(name="ps_c1", bufs=2, space="PSUM") as ps_c1,
            tc.tile_pool(name="ps_big", bufs=2, space="PSUM") as ps_big,
            tc.tile_pool(name="ps_o", bufs=2, space="PSUM") as ps_o,
        ):
            # ---- persistent SBUF tensors ----
            xT = persist.tile([128, nec, S], BF16)
            wqk = persist.tile([128, HPC, nec, 2 * DH], BF16)
            wv = persist.tile([128, nec, HD], BF16)
            wo = persist.tile([128, ncc, E], BF16)
            vS = persist.tile([128, nst, HPC * (DH + 1)], BF16)
            cT = persist.tile([128, ncc, S], BF16)
            mk = persist.tile([128, 2 * 1024], BF16)

            # ---- phase A: DMAs ordered so compute starts early ----
            # Scrub the exp-source PSUM banks: the narrowed diagonal score
            # blocks leave columns unwritten, and exp() of leftover garbage
            # from a previous NEFF can reach inf (then 0*inf=NaN in the
            # mask).  Zero once at startup; within-run staleness is bounded.
            for i in range(2):
                scrub = ps_big.tile([128, 1024], F32, tag="sbig",
                                    name=f"scrub{i}")
                nc.vector.memset(scrub, 0.0)
            # PE warmup: dummy matmuls during the input-DMA wait keep the
            # HAM clock gate from starting the real stream cold.
            warm = persist.tile([128, 512], BF16)
            nc.vector.memset(warm, 1.0)
            pwarm = ps_c1.tile([128, 512], F32, tag="c1")
            for i in range(32):
                nc.tensor.matmul(pwarm, lhsT=warm[:, 0:128], rhs=warm,
                                 start=(i == 0), stop=(i == 31))
            nc.vector.memset(vS, 1.0)
            nc.sync.dma_start(out=wqk[:, 0], in_=wqkt[:, 0])
            for j in range(nqt):
                nc.sync.dma_start(out=xT[:, :, j * 512:(j + 1) * 512],
                                  in_=xtd[:, :, j * 512:(j + 1) * 512])
                if j == 0:
                    nc.sync.dma_start(out=wv, in_=wvt[:, :, :])
            nc.sync.dma_start(out=wqk[:, 1:HPC], in_=wqkt[:, 1:HPC])
            nc.sync.dma_start(out=mk, in_=maskab[:, :])
            nc.sync.dma_start(out=wo, in_=wot[:, :, :])

            class C1Emitter:
                """q/k projection for one head, emitted one matmul at a
                time so the calls can be interleaved into the previous
                head's attention (fills PE bubbles while ACT runs exp)."""

                def __init__(self, h, q_sb, kt_sb):
                    self.h, self.q_sb, self.kt_sb = h, q_sb, kt_sb
                    self.sc = 0
                    self.ec = 0
                    self.pqk = None

                def emit_one(self):
                    if self.sc >= nqt:
                        return False
                    if self.pqk is None:
                        self.pqk = ps_c1.tile([128, 512], F32, tag="c1")
                    sc, ec = self.sc, self.ec
                    nc.tensor.matmul(
                        self.pqk, lhsT=wqk[:, self.h, ec],
                        rhs=xT[:, ec, sc * 512:(sc + 1) * 512],
                        start=(ec == 0), stop=(ec == nec - 1))
                    self.ec += 1
                    if self.ec == nec:
                        nc.vector.tensor_copy(
                            out=self.q_sb[:, sc * 512:(sc + 1) * 512],
                            in_=self.pqk[0:64, :])
                        nc.vector.tensor_copy(
                            out=self.kt_sb[:, sc * 512:(sc + 1) * 512],
                            in_=self.pqk[64:128, :])
                        self.pqk = None
                        self.ec = 0
                        self.sc += 1
                    return True

                def finish(self):
                    while self.emit_one():
                        pass

            def emit_c1(h, q_sb, kt_sb):
                C1Emitter(h, q_sb, kt_sb).finish()

            # head 0's projection first: needs only wqk[0] + xT chunks
            qk_tiles = [(qp.tile([64, S], BF16, tag="q", name="q0"),
                         kp.tile([64, S], BF16, tag="kt", name="kt0"))]
            emit_c1(0, *qk_tiles[0])

            # v projection per 128-token tile (stream wv columns)
            for j in range(nqt):
                for st in range(4 * j, 4 * j + 4):
                    pv = ps_c1.tile([128, HD], F32, tag="c1")
                    for ec in range(nec):
                        nc.tensor.matmul(
                            pv, lhsT=xT[:, ec, st * 128:(st + 1) * 128],
                            rhs=wv[:, ec], start=(ec == 0),
                            stop=(ec == nec - 1))
                    nc.vector.tensor_copy(
                        out=vS[:, st].rearrange("p (h m) -> p h m",
                                                m=DH + 1)[:, :, 0:DH],
                        in_=pv.rearrange("p (h m) -> p h m", m=DH))

            # ---- phase C: per head ----
            for h in range(HPC):
                q_sb, kt_sb = qk_tiles[h]
                if h + 1 < HPC:  # next head's projection, drip-fed into C2
                    qk_tiles.append(
                        (qp.tile([64, S], BF16, tag="q", name=f"q{h + 1}"),
                         kp.tile([64, S], BF16, tag="kt", name=f"kt{h + 1}")))
                    nxt = C1Emitter(h + 1, *qk_tiles[h + 1])
                else:
                    nxt = None

                # C2: attention per query tile
                for qt in range(nqt):
                    po = ps_o.tile([DH + 1, 512], F32, tag="o")
                    ngrp = 2 * qt + 2
                    pts = {}

                    def emit_scores(g):
                        ps2 = ps_big.tile([128, 1024], F32, tag="sbig")
                        for kk in range(2):
                            kt = 2 * g + kk
                            d = kt - 4 * qt
                            n0 = 128 * d if d > 0 else 0
                            nc.tensor.matmul(
                                ps2[:, kk * 512 + n0:(kk + 1) * 512],
                                lhsT=kt_sb[:, kt * 128:(kt + 1) * 128],
                                rhs=q_sb[:, qt * 512 + n0:(qt + 1) * 512],
                                start=True, stop=True)
                        pt = ptp.tile([128, 1024], BF16, tag="pt")
                        nc.scalar.activation(out=pt, in_=ps2, func=EXP,
                                             scale=0.125)
                        if g >= 2 * qt:  # diagonal groups need causal mask
                            mi = g - 2 * qt
                            nc.vector.tensor_mul(
                                pt, pt, mk[:, mi * 1024:(mi + 1) * 1024])
                        pts[g] = pt

                    def emit_av(g):
                        pt = pts.pop(g)
                        for kk in range(2):
                            kt = 2 * g + kk
                            d = kt - 4 * qt
                            n0 = 128 * d if d > 0 else 0
                            nc.tensor.matmul(
                                po[:, n0:512],
                                lhsT=vS[:, kt, h * (DH + 1):(h + 1) * (DH + 1)],
                                rhs=pt[:, kk * 512 + n0:(kk + 1) * 512],
                                start=(g == 0 and kk == 0),
                                stop=(g == ngrp - 1 and kk == 1),
                                skip_group_check=True)

                    # software pipeline: AV one group behind scores, with
                    # next-head projection matmuls drip-fed into the ACT
                    # bubbles (exp is the per-group rate limiter)
                    emit_scores(0)
                    for g in range(1, ngrp):
                        emit_scores(g)
                        emit_av(g - 1)
                        if nxt is not None:
                            nxt.emit_one()
                            nxt.emit_one()
                    emit_av(ngrp - 1)

                    # normalize: rec from PSUM sums row, gpsimd broadcast
                    sums = recp.tile([1, 512], F32, tag="sums")
                    nc.vector.tensor_copy(out=sums, in_=po[DH:DH + 1, :])
                    rec = recp.tile([1, 512], F32, tag="rec")
                    nc.vector.reciprocal_approx_fast(out=rec, in_=sums)
                    bc = bcp.tile([64, 512], F32, tag="bc")
                    nc.gpsimd.partition_broadcast(bc, rec)
                    nc.vector.tensor_mul(
                        cT[64 * (h % 2):64 * (h % 2) + 64, h // 2,
                           qt * 512:(qt + 1) * 512],
                        po[0:DH, :], bc)

                if nxt is not None:
                    nxt.finish()

            # ---- phase D: partial out-projection (bf16 out) ----
            for st in range(nst):
                pd = ps_big.tile([128, E], F32, tag="sbig")
                for c in range(ncc):
                    for n2 in range(2):
                        mm = nc.tensor.matmul(
                            pd[:, n2 * 512:(n2 + 1) * 512],
                            lhsT=cT[:, c, st * 128:(st + 1) * 128],
                            rhs=wo[:, c, n2 * 512:(n2 + 1) * 512],
                            start=(c == 0), stop=(c == ncc - 1),
                            skip_group_check=True)
                        if n2 == 1:  # same stationary as the n2=0 matmul
                            mm.ins.ldweights = False
                osb = outp.tile([128, E], BF16, tag="osb")
                nc.scalar.copy(osb, pd)
                nc.sync.dma_start(out=out[st * 128:(st + 1) * 128, :], in_=osb)

    nc.finalize()
    return nc


def _make_masks(NQ=512):
    """[128, 2*1024] bf16: two diag-group masks (d=0,1 | d=2,3)."""
    j = np.arange(128)[:, None]
    i = np.arange(NQ)[None, :]
    blocks = [(j <= i - 128 * d).astype(np.float32) for d in range(4)]
    mA = np.concatenate(blocks[0:2], axis=1)
    mB = np.concatenate(blocks[2:4], axis=1)
    return np.ascontiguousarray(
        np.concatenate([mA, mB], axis=1)).astype(BF)


def _host_prep(x, Wq, Wk, Wv, Wo, HPC=8, DH=64):
    """Build the 8 per-core input maps (bf16, pre-transposed)."""
    B, S, E = x.shape
    nec = E // 128
    HD = HPC * DH
    masks = _make_masks()
    xts = []
    for b in range(B):
        xt = x[b].T.reshape(nec, 128, S).transpose(1, 0, 2)
        xts.append(np.ascontiguousarray(xt).astype(BF))
    in_maps = []
    for c in range(N_CORES):
        b, g = c // 2, c % 2
        hs = slice(HPC * g, HPC * g + HPC)
        # [h, 2*DH, E] stacked q|k  ->  [128(e%), h, ec, 2*DH]
        wqk = np.concatenate([Wq[hs], Wk[hs]], axis=1)          # [HPC,128,E]
        wqk = wqk.transpose(2, 0, 1).reshape(nec, 128, HPC, 2 * DH)
        wqkt = np.ascontiguousarray(wqk.transpose(1, 2, 0, 3)).astype(BF)
        # Wv slice -> [128, ec, HD]
        wvt = Wv[hs].transpose(2, 0, 1).reshape(nec, 128, HD)
        wvt = np.ascontiguousarray(wvt.transpose(1, 0, 2)).astype(BF)
        # Wo columns slice, transposed -> [128, ncc, E]
        wot = np.ascontiguousarray(Wo[:, HD * g:HD * (g + 1)].T)  # [HD, E]
        wot = np.ascontiguousarray(
            wot.reshape(HD // 128, 128, E).transpose(1, 0, 2)).astype(BF)
        in_maps.append({
            "xt": xts[b],
            "wqkt": wqkt, "wvt": wvt, "wot": wot, "maskab": masks,
        })
    return in_maps


_NC_CACHE = {}


def kernel(x, Wq, Wk, Wv, Wo):
    x = np.asarray(x, dtype=np.float32)
    Wq = np.asarray(Wq, dtype=np.float32)
    Wk = np.asarray(Wk, dtype=np.float32)
    Wv = np.asarray(Wv, dtype=np.float32)
    Wo = np.asarray(Wo, dtype=np.float32)
    B, S, E = x.shape
    H, DH, _ = Wq.shape
    HPC = H // 2

    key = (S, E, HPC, DH)
    if key not in _NC_CACHE:
        _NC_CACHE[key] = build_nc(S=S, E=E, HPC=HPC, DH=DH)
    nc = _NC_CACHE[key]

    in_maps = _host_prep(x, Wq, Wk, Wv, Wo, HPC=HPC, DH=DH)
    res = run_bass_kernel_spmd(nc, in_maps, core_ids=list(range(N_CORES)))
    kernel.last_results = res

    out = np.empty((B, S, E), dtype=np.float32)
    for b in range(B):
        out[b] = (res.results[2 * b]["out"].astype(np.float32)
                  + res.results[2 * b + 1]["out"].astype(np.float32))
    return out
